# revision 1
# baseline (speedup 1.0000x reference)
"""Trainium2 Bass kernel for causal attention block (B=4, T=4096, D=256, k=v=64).

Sharding: 2 cores per batch (8 cores, 4 batches). Each core handles 4 q-chunks
of 512 rows, chosen with folded interleaving so causal work is balanced:
  parity0 -> chunks [7, 5, 2, 0]   (causal extents 8, 6, 3, 1 blocks of 512)
  parity1 -> chunks [6, 4, 3, 1]   (extents 7, 5, 4, 2)
The SPMD graph is identical on all cores: slot s processes SLOT_BLKS[s] =
[8, 6, 4, 2] s-blocks; cores whose chunk has a smaller extent get the last
block masked dead via a host-supplied 0/1 mask (also carries the diagonal
triangle masks).  All transposes are done host-side (inputs passed as X^T).

Per core on device:
  Q^T = Wq^T @ XqT,  K^T = Wk^T @ XkvT,  V^T = Wv^T @ XkvT   (bf16 matmuls)
  V natural via PE transpose; augmented with a ones column (fused rowsum).
  Per (slot, block):  S^T[s128x4, q512] = K^T-block^T-ish matmuls -> PSUM,
  P^T = exp(S^T / 8) (ScalarE, PSUM->SBUF bf16), mask on tail blocks,
  O^T[65, 512] += [V|1]^T-block @ P^T  (PSUM accumulate).
  Epilogue: transpose O^T, divide by rowsum, DMA out [2048, 64].
Host: scatters per-core rows back, concatenates with inputs.
"""

import numpy as np
import ml_dtypes

B, T, D, KS = 4, 4096, 256, 64
CH = 512
SLOT_BLKS = [8, 6, 4, 2]          # uniform graph geometry (s-blocks per slot)
CHUNKS = {0: [7, 5, 2, 0], 1: [6, 4, 3, 1]}   # parity -> chunk id per slot

_CACHE = {}


def _masks():
    """Return (exact, short) mask patterns, each [128, 2*4*512] bf16.

    Layout per pattern: tail(2) x sub(4) x 512 along free dim.
    exact  (chunk extent == slot size):  tail0 = FULL ones, tail1 = DIAG tri
    short  (extent == size - 1):         tail0 = DIAG tri,  tail1 = DEAD zeros
    DIAG tri for sub k: keep (=1) iff 128*k + p <= f.
    """
    p = np.arange(128)[:, None]
    f = np.arange(512)[None, :]
    diag = np.concatenate(
        [(128 * k + p <= f).astype(np.float32) for k in range(4)], axis=1
    )  # [128, 2048]
    ones = np.ones((128, 2048), np.float32)
    zeros = np.zeros((128, 2048), np.float32)
    exact = np.concatenate([ones, diag], axis=1).astype(ml_dtypes.bfloat16)
    short = np.concatenate([diag, zeros], axis=1).astype(ml_dtypes.bfloat16)
    return exact, short


def _build():
    import concourse.bass as bass
    import concourse.tile as tile
    from concourse import bacc, mybir

    f32 = mybir.dt.float32
    bf16 = mybir.dt.bfloat16
    FT = mybir.ActivationFunctionType

    nc = bacc.Bacc("TRN2", target_bir_lowering=False, debug=False, num_devices=8)

    d_xkvT = nc.dram_tensor("xkvT", [D, T], f32, kind="ExternalInput")
    d_xqT = nc.dram_tensor("xqT", [D, 4 * CH], f32, kind="ExternalInput")
    d_wk = nc.dram_tensor("wk", [D, KS], f32, kind="ExternalInput")
    d_wq = nc.dram_tensor("wq", [D, KS], f32, kind="ExternalInput")
    d_wv = nc.dram_tensor("wv", [D, KS], f32, kind="ExternalInput")
    d_mask = nc.dram_tensor("mask", [128, 8192], bf16, kind="ExternalInput")
    d_idb = nc.dram_tensor("idb", [128, 64], bf16, kind="ExternalInput")
    d_idf = nc.dram_tensor("idf", [128, 128], f32, kind="ExternalInput")
    d_out = nc.dram_tensor("out", [4 * CH, KS], f32, kind="ExternalOutput")

    from contextlib import ExitStack

    with tile.TileContext(nc) as tc, ExitStack() as ctx:
        const = ctx.enter_context(tc.tile_pool(name="const", bufs=1))
        xf = ctx.enter_context(tc.tile_pool(name="xf", bufs=1))
        xb = ctx.enter_context(tc.tile_pool(name="xb", bufs=1))
        kvq = ctx.enter_context(tc.tile_pool(name="kvq", bufs=1))
        ptp = ctx.enter_context(tc.tile_pool(name="ptp", bufs=4))
        otp = ctx.enter_context(tc.tile_pool(name="otp", bufs=1))
        finsb = ctx.enter_context(tc.tile_pool(name="finsb", bufs=2))
        rcp = ctx.enter_context(tc.tile_pool(name="rcp", bufs=4))
        outp = ctx.enter_context(tc.tile_pool(name="outp", bufs=1))

        # ---- constants ----
        w_b = {}
        for nm, dt_ in (("wk", d_wk), ("wq", d_wq), ("wv", d_wv)):
            tf = const.tile([128, 128], f32, name=nm + "f")
            nc.sync.dma_start(
                tf.rearrange("p (c k) -> p c k", k=KS),
                dt_.ap().rearrange("(c p) k -> p c k", p=128),
            )
            tb = const.tile([128, 128], bf16, name=nm + "b")
            nc.vector.tensor_copy(tb[:], tf[:])
            w_b[nm] = tb
        idb_sb = const.tile([128, 64], bf16, name="idb")
        nc.sync.dma_start(idb_sb[:], d_idb.ap())
        idf_sb = const.tile([128, 128], f32, name="idf")
        nc.sync.dma_start(idf_sb[:], d_idf.ap())

        # ---- raw inputs (two 128-partition halves of X^T side by side) ----
        # DMA and cast slices are aligned (per c-half, per t-window) so
        # projections can chase the DMAs slice by slice.
        xq_f = xf.tile([128, 4096], f32, name="xqf")
        xq_b = xb.tile([128, 4096], bf16, name="xqb")
        dq = d_xqT.ap().rearrange("(c p) t -> p c t", p=128)
        for c in range(2):
            nc.sync.dma_start(xq_f[:, 2048 * c:2048 * (c + 1)], dq[:, c, :])
            nc.vector.tensor_copy(xq_b[:, 2048 * c:2048 * (c + 1)],
                                  xq_f[:, 2048 * c:2048 * (c + 1)])
        xkv_f = xf.tile([128, 8192], f32, name="xkvf")
        xkv_b = xb.tile([128, 8192], bf16, name="xkvb")
        dk = d_xkvT.ap().rearrange("(c p) t -> p c t", p=128)
        for j in range(8):
            c, tw = j % 2, j // 2
            sl = slice(4096 * c + 1024 * tw, 4096 * c + 1024 * (tw + 1))
            nc.sync.dma_start(xkv_f[:, sl], dk[:, c, 1024 * tw:1024 * (tw + 1)])
            nc.vector.tensor_copy(xkv_b[:, sl], xkv_f[:, sl])
        mask_sb = const.tile([128, 8192], bf16, name="mask")
        nc.sync.dma_start(mask_sb[:], d_mask.ap())

        # ---- projections ----
        # kvT: partitions 0:64 = K^T [64, 4096], partitions 64:128 = V^T
        kvT = kvq.tile([128, T], bf16, name="kvT")
        qT = kvq.tile([64, 4 * CH], bf16, name="qT")
        # duplicates of K^T / Q^T in partitions 64:128 so odd score matmuls can
        # row-tile at tile_position (64,0) (walrus: stationary base == row pos)
        kq2 = kvq.tile([128, T + 4 * CH], bf16, name="kq2")
        k2 = kq2[64:128, 0:T]
        q2 = kq2[64:128, T:T + 4 * CH]
        v_aug = kvq.tile([128, 32 * 65], bf16, name="vaug")
        v_re = v_aug.rearrange("p (n w) -> p n w", w=65)

        nc.gpsimd.memset(v_re[:, :, 64:65], 1.0)

        # ---- main attention loop (projections interleaved into slot 0) ----
        oT = otp.tile([65, 4 * 512], f32, name="oT")
        out_sb = outp.tile([128, 1024], f32, name="outsb")
        d_out_r = d_out.ap().rearrange("(n p) v -> p n v", p=128)
        out_sb_r = out_sb.rearrange("p (n v) -> p n v", v=KS)

        def emit_final(slot_):
            # transpose oT[:, slot] -> [q, 65], normalize, store slot rows
            fp = finp.tile([128, 512], f32, name="finps", tag="pvfin")[:, 0:260]
            for k in range(4):
                nc.tensor.transpose(
                    fp[:, 65 * k:65 * (k + 1)],
                    oT[:, 512 * slot_ + 128 * k:512 * slot_ + 128 * (k + 1)],
                    idf_sb[0:65, 0:65])
            ff = finsb.tile([128, 260], f32, name="ff")
            nc.vector.tensor_copy(ff[:], fp[:])
            ffr = ff.rearrange("p (k w) -> p k w", w=65)
            rc = rcp.tile([128, 4], f32, name="rc")
            nc.vector.reciprocal(rc[:], ffr[:, :, 64])
            for k in range(4):
                piece = 4 * slot_ + k
                nc.vector.tensor_scalar_mul(
                    out_sb[:, 64 * piece:64 * (piece + 1)],
                    ff[:, 65 * k:65 * k + 64], rc[:, k:k + 1])
            nc.sync.dma_start(d_out_r[:, 4 * slot_:4 * (slot_ + 1), :],
                              out_sb_r[:, 4 * slot_:4 * (slot_ + 1), :])

        with tc.tile_pool(name="ringp", bufs=3, space="PSUM") as ringp, \
             tc.tile_pool(name="pvfin", bufs=2, space="PSUM") as pvp:
            finp = pvp

            def emit_qproj(j):
                # q-slots (2j, 2j+1) packed into one psum tile's partition halves
                ps = ringp.tile([128, 1024], f32, name="projq", tag="ring")
                for half in range(2):
                    for ci in range(2):
                        jj = 2 * j + half
                        nc.tensor.matmul(
                            ps[64 * half:64 * (half + 1), 0:512],
                            w_b["wq"][:, 64 * ci:64 * ci + 64],
                            xq_b[:, 2048 * ci + 512 * jj:2048 * ci + 512 * (jj + 1)],
                            start=(ci == 0), stop=(ci == 1))
                nc.vector.tensor_copy(qT[:, 1024 * j:1024 * j + 512], ps[0:64, 0:512])
                nc.vector.tensor_copy(qT[:, 1024 * j + 512:1024 * (j + 1)],
                                      ps[64:128, 0:512])

            def emit_kv(j):
                ps = ringp.tile([128, 1024], f32, name="projkv", tag="ring")
                rhs0 = xkv_b[:, 512 * j:512 * (j + 1)]
                rhs1 = xkv_b[:, 4096 + 512 * j:4096 + 512 * (j + 1)]
                nc.tensor.matmul(ps[0:64, 0:512], w_b["wk"][:, 0:64], rhs0, start=True, stop=False)
                nc.tensor.matmul(ps[0:64, 0:512], w_b["wk"][:, 64:128], rhs1, start=False, stop=True)
                nc.tensor.matmul(ps[64:128, 0:512], w_b["wv"][:, 0:64], rhs0, start=True, stop=False)
                nc.tensor.matmul(ps[64:128, 0:512], w_b["wv"][:, 64:128], rhs1, start=False, stop=True)
                nc.vector.tensor_copy(kvT[:, 512 * j:512 * (j + 1)], ps[:, 0:512])

            def emit_vtrans(g):
                # V^T -> V natural for s-subblocks 8g..8g+7
                vp = ringp.tile([128, 1024], bf16, name="vtps", tag="ring")
                for k in range(8):
                    i = 8 * g + k
                    nc.tensor.transpose(
                        vp[:, 64 * k:64 * (k + 1)],
                        kvT[64:128, 128 * i:128 * (i + 1)],
                        idb_sb[64:128, :])
                nc.vector.tensor_copy(
                    v_re[:, 8 * g:8 * (g + 1), 0:64],
                    vp[:, 0:512].rearrange("p (n w) -> p n w", w=64))

            emit_qproj(0)
            emit_qproj(1)
            tile_idx = 0
            # PV work deferred by TWO tiles: by the time PV(k-2) is issued on
            # PE, its exp/mask deps are two ACT-periods old, so the in-order
            # PE stream never stalls inside PV, and the next tile's score
            # matmuls (which feed ACT) issue early.
            from collections import deque
            pending = deque()

            def emit_pv(p):
                pt_, slot_, blk_, ov_, nblk_ = p
                for s in range(4):
                    sb = 4 * blk_ + s
                    nc.tensor.matmul(
                        ov_[:], v_aug[:, 65 * sb:65 * (sb + 1)],
                        pt_[:, 512 * s:512 * (s + 1)],
                        start=(blk_ == 0 and s == 0),
                        stop=(blk_ == nblk_ - 1 and s == 3))
                if blk_ == nblk_ - 1:
                    nc.vector.tensor_copy(
                        oT[:, 512 * slot_:512 * (slot_ + 1)], ov_[:])
                    emit_final(slot_)

            for slot in range(4):
                nblk = SLOT_BLKS[slot]
                if slot == 1:
                    # bulk-duplicate K^T/Q^T into partitions 64:128 (cheap
                    # 2x-mode SBUF copies) for row-tiled scores in slots 1-3
                    nc.vector.tensor_copy(k2[:], kvT[0:64, :])
                    nc.vector.tensor_copy(q2[:], qT[:])
                ov = pvp.tile([128, 512], f32, name="ovps", tag="pvfin")[0:65, :]
                for blk in range(nblk):
                    if slot == 0 and blk % 2 == 0:
                        emit_kv(blk)
                        emit_kv(blk + 1)
                        emit_vtrans(blk // 2)
                    pt = ptp.tile([128, 2048], bf16, name="pt")
                    # two bank-pair score tiles per (slot, blk); separate pool
                    # tiles give exact per-pair dependency tracking (exp of a
                    # pair waits only on its own two matmuls).
                    for h in range(2):
                        rg = ringp.tile([128, 1024], f32, name="ring", tag="ring")
                        for s in (2 * h, 2 * h + 1):
                            sb = 4 * blk + s
                            if s % 2 == 0 or slot == 0:
                                nc.tensor.matmul(
                                    rg[:, 512 * (s - 2 * h):512 * (s - 2 * h + 1)],
                                    kvT[0:64, 128 * sb:128 * (sb + 1)],
                                    qT[:, 512 * slot:512 * (slot + 1)],
                                    start=True, stop=True)
                            else:
                                # concurrent row-tile in array rows 64:127
                                nc.tensor.matmul(
                                    rg[:, 512 * (s - 2 * h):512 * (s - 2 * h + 1)],
                                    k2[:, 128 * sb:128 * (sb + 1)],
                                    q2[:, 512 * slot:512 * (slot + 1)],
                                    start=True, stop=True,
                                    tile_position=(64, 0))
                        nc.scalar.activation(pt[:, 1024 * h:1024 * (h + 1)],
                                             rg[:], FT.Exp, scale=0.125)
                    if blk >= nblk - 2:
                        tail = blk - (nblk - 2)
                        moff = 4096 * (slot // 2) + 2048 * tail
                        nc.vector.tensor_mul(pt[:], pt[:], mask_sb[:, moff:moff + 2048])
                    pending.append((pt, slot, blk, ov, nblk))
                    if len(pending) > 2:
                        emit_pv(pending.popleft())
                    tile_idx += 1
            while pending:
                emit_pv(pending.popleft())

    nc.compile()
    return nc


def _get_nc():
    if "nc" not in _CACHE:
        _CACHE["nc"] = _build()
    return _CACHE["nc"]


def kernel(inputs, key_w, query_w, value_w):
    from concourse.bass_utils import run_bass_kernel_spmd

    inputs = np.asarray(inputs, np.float32)
    key_w = np.asarray(key_w, np.float32)
    query_w = np.asarray(query_w, np.float32)
    value_w = np.asarray(value_w, np.float32)

    exact, short = _masks()
    masks = {
        0: np.ascontiguousarray(np.concatenate([exact, short], axis=1)),
        1: np.ascontiguousarray(np.concatenate([short, exact], axis=1)),
    }
    idb = np.zeros((128, 64), ml_dtypes.bfloat16)
    for p in range(128):
        idb[p, p % 64] = 1
    idf = np.eye(128, dtype=np.float32)

    in_maps = []
    for c in range(8):
        b, par = c // 2, c % 2
        xT = np.ascontiguousarray(inputs[b].T)  # [256, 4096]
        rows = np.concatenate(
            [np.arange(CH * ch, CH * (ch + 1)) for ch in CHUNKS[par]])
        xqT = np.ascontiguousarray(inputs[b][rows].T)  # [256, 2048]
        in_maps.append({
            "xkvT": xT, "xqT": xqT,
            "wk": key_w, "wq": query_w, "wv": value_w,
            "mask": masks[par], "idb": idb, "idf": idf,
        })

    nc = _get_nc()
    _CACHE["last_in_maps"] = in_maps
    res = run_bass_kernel_spmd(nc, in_maps, core_ids=list(range(8))).results

    out = np.empty((B, T, D + KS), np.float32)
    out[:, :, :D] = inputs
    for c in range(8):
        b, par = c // 2, c % 2
        r = res[c]["out"] if isinstance(res[c], dict) else res[c]
        rows = np.concatenate(
            [np.arange(CH * ch, CH * (ch + 1)) for ch in CHUNKS[par]])
        out[b, rows, D:] = np.asarray(r, np.float32)
    return out

